# revision 7
# baseline (speedup 1.0000x reference)
"""Trainium2 Bass kernel for LFPNetMC: residual conv1d blocks + 1x1 conv + 2-layer MLP.

Reference computation (per batch row b, fp32):
    y  = block(x) + x;  y = block(y) + y          (block = conv5->relu->conv3->relu, shared weights)
    y  = relu(ck1_w * y + ck1_b)                  (1x1 conv)
    z  = relu(y @ fc1_w.T + fc1_b)                (L=4096 -> H=2048)
    out= z @ fc2_w.T + fc2_b                      (H=2048 -> OUT=1024)

Strategy: data-parallel over 8 NeuronCores (512 batch rows each). On each core
everything lives in [L partitions, batch free] layout so the convs become banded
lhsT matmuls on the tensor engine (the contraction dim = L sits on partitions),
and fc1/fc2 are standard PSUM-accumulated matmuls. Host packs weights so each
DMA is one contiguous read.
"""

import os

if os.environ.get("JAX_PLATFORMS") == "cpu":
    # run_bass_via_pjrt needs the axon NeuronCore platform visible.
    os.environ["JAX_PLATFORMS"] = ""

import numpy as np

import concourse.mybir as mybir
import concourse.tile as tile
from concourse import bacc, bass_utils

F32 = mybir.dt.float32
AF = mybir.ActivationFunctionType

B, L, H, OUT = 4096, 4096, 2048, 1024
NCORES = 8
BC = B // NCORES          # 512 batch rows per core
VALID = 116               # valid conv output rows per L-tile (128 - 2*6 halo)
TIN = 128                 # input rows per L-tile
MARGIN = 6                # halo on each side (2+1+2+1)
NT = (L + VALID - 1) // VALID   # 36 L-tiles
LPAD = VALID * NT         # 4176
PADROWS = VALID * (NT - 1) + TIN  # 4188
NH1 = H // 128            # 16 fc1 output tiles
NH2 = H // 128            # 16 fc2 contraction chunks
NO = OUT // 128           # 8 fc2 output tiles

# wc packing: column offsets of the banded/shift matrices in the [128, 768] tensor
_WC_COLS = {"WA1": 0, "WA2": 128, "WB1": 256, "WB2": 384, "S3": 512, "S3b": 640}
# (K, M) of each banded matrix
_WC_SHAPES = {
    "WA1": (128, 124), "WA2": (124, 122), "WB1": (122, 118),
    "WB2": (118, 116), "S3": (128, 122), "S3b": (122, 116),
}

_CACHE = {}


def _build_program():
    """Build + compile the Bass program. Input-value independent."""
    nc = bacc.Bacc("TRN2", target_bir_lowering=False, debug=False,
                   num_devices=NCORES)

    xt_d = nc.dram_tensor("xt", [PADROWS, BC], F32, kind="ExternalInput").ap()
    wc_d = nc.dram_tensor("wc", [128, 768], F32, kind="ExternalInput").ap()
    scal_d = nc.dram_tensor("scal", [128, 20], F32, kind="ExternalInput").ap()
    w1_d = nc.dram_tensor("w1", [NH1, VALID, NT * 128], F32, kind="ExternalInput").ap()
    b1_d = nc.dram_tensor("b1", [128, NH1], F32, kind="ExternalInput").ap()
    w2_d = nc.dram_tensor("w2", [NO, 128, NH2 * 128], F32, kind="ExternalInput").ap()
    b2_d = nc.dram_tensor("b2", [128, NO], F32, kind="ExternalInput").ap()
    out_d = nc.dram_tensor("out", [NO, 128, BC], F32, kind="ExternalOutput").ap()

    def wcs(name):
        k, m = _WC_SHAPES[name]
        c = _WC_COLS[name]
        return (slice(0, k), slice(c, c + m))

    with tile.TileContext(nc) as tc:
        with (
            tc.tile_pool(name="const", bufs=1) as cpool,
            tc.tile_pool(name="xin", bufs=3) as xpool,
            tc.tile_pool(name="conv", bufs=2) as vpool,
            tc.tile_pool(name="big", bufs=1) as bigpool,
            tc.tile_pool(name="w1p", bufs=2) as w1pool,
            tc.tile_pool(name="w2p", bufs=2) as w2pool,
            tc.tile_pool(name="outp", bufs=2) as opool,
            tc.tile_pool(name="psum", bufs=2, space="PSUM") as pp,
        ):
            wc_sb = cpool.tile([128, 768], F32)
            nc.sync.dma_start(wc_sb[:], wc_d[:])
            scal_sb = cpool.tile([128, 20], F32)
            nc.sync.dma_start(scal_sb[:], scal_d[:])
            b1_sb = cpool.tile([128, NH1], F32)
            nc.sync.dma_start(b1_sb[:], b1_d[:])
            b2_sb = cpool.tile([128, NO], F32)
            nc.sync.dma_start(b2_sb[:], b2_d[:])

            y5 = bigpool.tile([VALID, NT * BC], F32)   # conv-stack output, [l, b]
            z = bigpool.tile([128, NH1 * BC], F32)     # fc1 output, [h, b]

            WA1, WA2 = wc_sb[wcs("WA1")], wc_sb[wcs("WA2")]
            WB1, WB2 = wc_sb[wcs("WB1")], wc_sb[wcs("WB2")]
            S3, S3b = wc_sb[wcs("S3")], wc_sb[wcs("S3b")]
            # scal columns: 0..3 = cb_b1, cb_b2, ck1_b, ck1_w (broadcast);
            # 4..9 / 10..15 = boundary-tile (mask, bias*mask) pairs for the
            # y1/y2/y3 stages of tile 0 / tile NT-1. Masked scale+bias zeroes
            # the rows whose absolute l falls outside [0, L) — the reference
            # zero-pads every conv layer, so phantoms must not propagate.
            def sb(t, stage, p, base_col):
                """(scale, bias) for the stage's ACT evac on tile t."""
                if t == 0:
                    c = 4 + 2 * stage
                    return scal_sb[0:p, c:c + 1], scal_sb[0:p, c + 1:c + 2]
                if t == NT - 1:
                    c = 10 + 2 * stage
                    return scal_sb[0:p, c:c + 1], scal_sb[0:p, c + 1:c + 2]
                return 1.0, scal_sb[0:p, base_col:base_col + 1]

            bias_k1 = scal_sb[0:VALID, 2:3]         # ck1_b
            scale_k1 = scal_sb[0:VALID, 3:4]        # ck1_w

            # ---------------- conv stack, one L-tile at a time ----------------
            for t in range(NT):
                abs0 = VALID * t  # absolute l of first valid output row
                xt_t = xpool.tile([TIN, BC], F32, tag="xt")
                nc.sync.dma_start(xt_t[:], xt_d[abs0:abs0 + TIN, :])

                ps1 = pp.tile([124, BC], F32, tag="psA")
                nc.tensor.matmul(ps1[:], WA1, xt_t[:], start=True, stop=True)
                y1 = vpool.tile([124, BC], F32, tag="y1")
                s, b = sb(t, 0, 124, 0)
                nc.scalar.activation(y1[:], ps1[:], AF.Relu, bias=b, scale=s)

                ps2 = pp.tile([122, BC], F32, tag="psB")
                nc.tensor.matmul(ps2[:], WA2, y1[:], start=True, stop=True)
                t2 = vpool.tile([122, BC], F32, tag="t2")
                s, b = sb(t, 1, 122, 1)
                nc.scalar.activation(t2[:], ps2[:], AF.Relu, bias=b, scale=s)
                psx = pp.tile([122, BC], F32, tag="psS")
                nc.tensor.matmul(psx[:], S3, xt_t[:], start=True, stop=True)
                y2 = vpool.tile([122, BC], F32, tag="y2")
                nc.vector.tensor_add(y2[:], t2[:], psx[:])

                ps3 = pp.tile([118, BC], F32, tag="psA")
                nc.tensor.matmul(ps3[:], WB1, y2[:], start=True, stop=True)
                y3 = vpool.tile([118, BC], F32, tag="y3")
                s, b = sb(t, 2, 118, 0)
                nc.scalar.activation(y3[:], ps3[:], AF.Relu, bias=b, scale=s)

                ps4 = pp.tile([VALID, BC], F32, tag="psB")
                nc.tensor.matmul(ps4[:], WB2, y3[:], start=True, stop=True)
                t4 = vpool.tile([VALID, BC], F32, tag="t4")
                nc.scalar.activation(t4[:], ps4[:], AF.Relu,
                                     bias=scal_sb[0:VALID, 1:2])
                psy = pp.tile([VALID, BC], F32, tag="psS")
                nc.tensor.matmul(psy[:], S3b, y2[:], start=True, stop=True)
                y4 = vpool.tile([VALID, BC], F32, tag="y4")
                nc.vector.tensor_add(y4[:], t4[:], psy[:])

                # 1x1 conv + relu straight into the fc1 input buffer
                nc.scalar.activation(y5[:, t * BC:(t + 1) * BC], y4[:],
                                     AF.Relu, bias=bias_k1, scale=scale_k1)

            # ---------------- fc1: z = relu(W1 @ y5 + b1) ----------------
            for i in range(NH1):
                w1t = w1pool.tile([VALID, NT * 128], F32, tag="w1t")
                nc.sync.dma_start(w1t[:], w1_d[i])
                psf = pp.tile([128, BC], F32, tag="psF")
                for j in range(NT):
                    nc.tensor.matmul(
                        psf[:], w1t[:, j * 128:(j + 1) * 128],
                        y5[:, j * BC:(j + 1) * BC],
                        start=(j == 0), stop=(j == NT - 1),
                    )
                nc.scalar.activation(z[:, i * BC:(i + 1) * BC], psf[:],
                                     AF.Relu, bias=b1_sb[:, i:i + 1])

            # ---------------- fc2: out = W2 @ z + b2 ----------------
            for i2 in range(NO):
                w2t = w2pool.tile([128, NH2 * 128], F32, tag="w2t")
                nc.sync.dma_start(w2t[:], w2_d[i2])
                psf = pp.tile([128, BC], F32, tag="psF")
                for j2 in range(NH2):
                    nc.tensor.matmul(
                        psf[:], w2t[:, j2 * 128:(j2 + 1) * 128],
                        z[:, j2 * BC:(j2 + 1) * BC],
                        start=(j2 == 0), stop=(j2 == NH2 - 1),
                    )
                osb = opool.tile([128, BC], F32, tag="osb")
                nc.scalar.activation(osb[:], psf[:], AF.Identity,
                                     bias=b2_sb[:, i2:i2 + 1])
                nc.sync.dma_start(out_d[i2], osb[:])

    nc.compile()
    return nc


def _band(taps, k, m, shift=None):
    """W[r, c] nonzero iff r-c in [0, len(taps)) with coeff taps[r-c]
    (or iff r-c == shift for identity-shift matrices)."""
    w = np.zeros((k, m), np.float32)
    r = np.arange(k)[:, None]
    c = np.arange(m)[None, :]
    if shift is not None:
        w[(r - c) == shift] = 1.0
        return w
    d = r - c
    for i, tap in enumerate(taps):
        w[d == i] = float(tap)
    return w


def _host_pack(inputs):
    """Host-side input packing -> per-core in_maps."""
    f = lambda a: np.ascontiguousarray(np.asarray(a), dtype=np.float32)
    x = f(inputs["x"]).reshape(B, L)
    w5 = f(inputs["cb_w1"]).reshape(5)
    w3 = f(inputs["cb_w2"]).reshape(3)
    cb_b1 = float(np.asarray(inputs["cb_b1"]).reshape(()))
    cb_b2 = float(np.asarray(inputs["cb_b2"]).reshape(()))
    ck1_w = float(np.asarray(inputs["ck1_w"]).reshape(()))
    ck1_b = float(np.asarray(inputs["ck1_b"]).reshape(()))
    fc1_w = f(inputs["fc1_w"])
    fc1_b = f(inputs["fc1_b"])
    fc2_w = f(inputs["fc2_w"])
    fc2_b = f(inputs["fc2_b"])

    wc = np.zeros((128, 768), np.float32)

    def put(name, mat):
        k, m = _WC_SHAPES[name]
        assert mat.shape == (k, m)
        wc[0:k, _WC_COLS[name]:_WC_COLS[name] + m] = mat

    put("WA1", _band(w5, 128, 124))
    put("WA2", _band(w3, 124, 122))
    put("WB1", _band(w5, 122, 118))
    put("WB2", _band(w3, 118, 116))
    put("S3", _band(None, 128, 122, shift=3))
    put("S3b", _band(None, 122, 116, shift=3))

    scal = np.zeros((128, 20), np.float32)
    scal[:, 0], scal[:, 1], scal[:, 2], scal[:, 3] = cb_b1, cb_b2, ck1_b, ck1_w
    # boundary masks: (stage offset, conv bias) for y1/y2/y3 evacs
    stage_bias = [cb_b1, cb_b2, cb_b1]
    stage_off = [4, 3, 1]  # y1 abs l = abs0-4+idx, y2: -3, y3: -1
    for col0, t in ((4, 0), (10, NT - 1)):
        abs0 = VALID * t
        for s in range(3):
            start = abs0 - stage_off[s]
            idx = np.arange(128)
            mask = ((start + idx >= 0) & (start + idx < L)).astype(np.float32)
            scal[:, col0 + 2 * s] = mask
            scal[:, col0 + 2 * s + 1] = stage_bias[s] * mask

    w1p = np.zeros((H, LPAD), np.float32)
    w1p[:, :L] = fc1_w
    # w1[i, k, t*128+c] = fc1_w[128i+c, 116t+k]
    w1d = np.ascontiguousarray(
        w1p.reshape(NH1, 128, NT, VALID).transpose(0, 3, 2, 1).reshape(NH1, VALID, NT * 128)
    )
    b1d = np.ascontiguousarray(fc1_b.reshape(NH1, 128).T)
    # w2[i2, p, j2*128+c] = fc2_w[128*i2+c, 128*j2+p]
    w2d = np.ascontiguousarray(
        fc2_w.reshape(NO, 128, NH2, 128).transpose(0, 3, 2, 1).reshape(NO, 128, NH2 * 128)
    )
    b2d = np.ascontiguousarray(fc2_b.reshape(NO, 128).T)

    in_maps = []
    for c in range(NCORES):
        xtpad = np.zeros((PADROWS, BC), np.float32)
        xtpad[MARGIN:MARGIN + L, :] = x[c * BC:(c + 1) * BC, :].T
        in_maps.append({
            "xt": xtpad, "wc": wc, "scal": scal,
            "w1": w1d, "b1": b1d, "w2": w2d, "b2": b2d,
        })
    return in_maps


def _get_nc():
    if "nc" not in _CACHE:
        _CACHE["nc"] = _build_program()
    return _CACHE["nc"]


def _run(inputs, **kwargs):
    nc = _get_nc()
    in_maps = _host_pack(inputs)
    res = bass_utils.run_bass_kernel_spmd(
        nc, in_maps, core_ids=list(range(NCORES)), **kwargs
    )
    out = np.empty((B, 1, OUT), np.float32)
    for c in range(NCORES):
        oc = res.results[c]["out"].reshape(OUT, BC)
        out[c * BC:(c + 1) * BC, 0, :] = oc.T
    return out, res


def kernel(**inputs) -> np.ndarray:
    out, _ = _run(inputs)
    return out
